# revision 9
# baseline (speedup 1.0000x reference)
"""Trainium2 Bass kernel for nn_CausalBankModel (decay-bank LM head).

Strategy (8 NeuronCores, vocab-tensor-parallel):
  - Every core computes the shared trunk (embedding gather on host, mode
    projection, decay-bank scan, both hidden layers) redundantly (~25% of
    the FLOPs); the two big [2048,1024]@[1024,4096] readout matmuls are
    sharded over vocab: core c owns columns [c*4000, (c+1)*4000).
  - Readout is processed in 4 groups of 4 token-tiles. Logit tiles stay
    RESIDENT IN SBUF (no DRAM round-trip). Per-group partial gate stats
    (sum/sumsq/max over the local vocab slice) are exchanged with one
    small AllGather per group; the gate + mixture for that group then
    runs from SBUF, overlapped with the next group's matmuls.
  - Stats work is split between ScalarE (PSUM->SBUF copy + sum accum) and
    VectorE (square+sumsq, max) to keep both far below TensorE's span.

Layouts on device (partition dim first):
  - xT    : [128(d), dh, b, 7+S] transposed embeddings, 7 zero cols pad
  - statesT: [128(m), mt, b, S]  decay-bank states (tensor_tensor_scan)
  - hT/h2T: [128(hidden), kt, B*S] bf16, lhsT of the readout matmuls
  - logits: [128(token), 2(br), 4096(vocab)] bf16, SBUF-resident per tile
"""

import os
import sys

import numpy as np

for _p in ("/opt/trn_rl_repo", "/opt/pypackages"):
    if _p not in sys.path and os.path.isdir(_p):
        sys.path.append(_p)

import ml_dtypes  # noqa: E402

from concourse import bacc, bass, tile  # noqa: E402
from concourse import mybir  # noqa: E402
from concourse.bass_utils import run_bass_kernel_spmd  # noqa: E402

F32 = mybir.dt.float32
F32R = mybir.dt.float32r
BF16 = mybir.dt.bfloat16
ALU = mybir.AluOpType
ACTF = mybir.ActivationFunctionType
AXX = mybir.AxisListType.X

V = 32000
D = 256
M = 256
W = 8
HL = 1024  # hidden width (both branches)
B = 2
S = 1024
BS = B * S            # 2048 tokens
NCORE = 8
VSH = V // NCORE      # 4000 true vocab cols per core
VPAD = 4096           # padded slice width
NVC = VPAD // 512     # 8 vocab chunks of 512
NT = BS // 128        # 16 token tiles
SP = S + W - 1        # 1031, padded time length
GSZ = 4               # token tiles per stats group
NG = NT // GSZ        # 4 groups

LAST_RESULT = None


def build(nc, with_vocab_bias=True):
    din = {}

    def inp(name, shape, dt):
        din[name] = nc.dram_tensor(name, list(shape), dt, kind="ExternalInput")
        return din[name]

    xt_d = inp("xt", [128, 2 * B * SP], F32R)
    inproj_d = inp("inproj", [D, M], F32R)
    decb_d = inp("decb", [M, 512], F32)
    w1_d = inp("w1", [M + D, HL], F32R)
    b1r_d = inp("b1r", [128, HL // 128], F32)
    lw1_d = inp("lw1", [W * D, HL], F32R)
    lb1r_d = inp("lb1r", [128, HL // 128], F32)
    wb_d = inp("wb", [HL, 2, VPAD], BF16)     # stacked [w2 | lw2]
    gwb_d = inp("gwb", [128, 6], F32)
    gbb_d = inp("gbb", [128, 1], F32)
    if with_vocab_bias:
        bb_d = inp("bb", [1, 2, VPAD], BF16)  # stacked [b2 | lb2]
        ones_d = inp("ones", [1, 128], BF16)

    out_d = nc.dram_tensor("out", [BS, VSH], BF16, kind="ExternalOutput")

    with tile.TileContext(nc) as tc:
        with (
            tc.tile_pool(name="cst", bufs=1) as cst,
            tc.tile_pool(name="ps", bufs=8, space=bass.MemorySpace.PSUM) as psp,
            tc.tile_pool(name="dram", bufs=1, space="DRAM") as drp,
        ):
            # per-(ti,vc) raw stats + gate consts (tiny, long-lived)
            ssum = [cst.tile([128, NT * NVC], F32, name=f"ssum{i}") for i in range(2)]
            ssq = [cst.tile([128, NT * NVC], F32, name=f"ssq{i}") for i in range(2)]
            smax = [cst.tile([128, NT * NVC], BF16, name=f"smax{i}") for i in range(2)]
            gwb_sb = cst.tile([128, 6], F32)
            nc.sync.dma_start(gwb_sb[:], gwb_d[:, :])
            gbb_sb = cst.tile([128, 1], F32)
            nc.sync.dma_start(gbb_sb[:], gbb_d[:, :])
            g_sb = cst.tile([128, NT], F32)

            with tc.tile_pool(name="ph", bufs=1) as php:  # spans trunk + readout
                hT = php.tile([128, 8, BS], BF16)
                h2T = php.tile([128, 8, BS], BF16)

                # ================= phase A: shared trunk =================
                with tc.tile_pool(name="pa", bufs=1) as pap:
                    xT = pap.tile([128, 2, B, SP], F32R)
                    for dh in range(2):
                        for b in range(B):
                            nc.sync.dma_start(
                                xT[:, dh, b, :],
                                xt_d[:, (dh * B + b) * SP:(dh * B + b + 1) * SP])

                    inproj_sb = pap.tile([128, 2, M], F32R)
                    for kt in range(2):
                        nc.sync.dma_start(inproj_sb[:, kt, :],
                                          inproj_d[kt * 128:(kt + 1) * 128, :])
                    decb_sb = pap.tile([128, 2, 512], F32)
                    for mt in range(2):
                        nc.sync.dma_start(decb_sb[:, mt, :],
                                          decb_d[mt * 128:(mt + 1) * 128, :])
                    b1r_sb = pap.tile([128, 8], F32)
                    nc.sync.dma_start(b1r_sb[:], b1r_d[:, :])
                    lb1r_sb = pap.tile([128, 8], F32)
                    nc.sync.dma_start(lb1r_sb[:], lb1r_d[:, :])
                    w1_sb = pap.tile([128, 4, HL], F32R)
                    for kt in range(4):
                        nc.sync.dma_start(w1_sb[:, kt, :],
                                          w1_d[kt * 128:(kt + 1) * 128, :])
                    # hoisted so the 8MB load overlaps the scan + w1 matmuls
                    lw1_sb = pap.tile([128, 16, HL], F32R)
                    for kt in range(16):
                        nc.sync.dma_start(lw1_sb[:, kt, :],
                                          lw1_d[kt * 128:(kt + 1) * 128, :])

                    statesT = pap.tile([128, 2, B, S], F32R)
                    for mt in range(2):
                        for b in range(B):
                            for hf in range(2):
                                ps = psp.tile([128, 512], F32)
                                for kt in range(2):
                                    nc.tensor.matmul(
                                        ps[:],
                                        inproj_sb[:, kt, mt * 128:(mt + 1) * 128],
                                        xT[:, kt, b,
                                           W - 1 + hf * 512:W - 1 + hf * 512 + 512],
                                        start=(kt == 0), stop=(kt == 1),
                                    )
                                init = (0.0 if hf == 0 else
                                        statesT[:, mt, b, hf * 512 - 1:hf * 512])
                                nc.vector.tensor_tensor_scan(
                                    statesT[:, mt, b, hf * 512:hf * 512 + 512],
                                    decb_sb[:, mt, :], ps[:], init,
                                    ALU.mult, ALU.add,
                                )

                    for hl in range(8):
                        for ch in range(4):
                            b, hf = ch // 2, ch % 2
                            ps = psp.tile([128, 512], F32)
                            for kt in range(4):
                                if kt < 2:
                                    rhs = statesT[:, kt, b, hf * 512:hf * 512 + 512]
                                else:
                                    rhs = xT[:, kt - 2, b,
                                             W - 1 + hf * 512:W - 1 + hf * 512 + 512]
                                nc.tensor.matmul(
                                    ps[:], w1_sb[:, kt, hl * 128:(hl + 1) * 128],
                                    rhs, start=(kt == 0), stop=(kt == 3),
                                )
                            nc.scalar.activation(
                                hT[:, hl, ch * 512:(ch + 1) * 512], ps[:],
                                ACTF.Relu, bias=b1r_sb[:, hl:hl + 1])

                    for hl in range(8):
                        for ch in range(4):
                            b, hf = ch // 2, ch % 2
                            ps = psp.tile([128, 512], F32)
                            for ki in range(16):
                                w, dh = ki // 2, ki % 2
                                rhs = xT[:, dh, b, hf * 512 + w:hf * 512 + w + 512]
                                nc.tensor.matmul(
                                    ps[:], lw1_sb[:, ki, hl * 128:(hl + 1) * 128],
                                    rhs, start=(ki == 0), stop=(ki == 15),
                                )
                            nc.scalar.activation(
                                h2T[:, hl, ch * 512:(ch + 1) * 512], ps[:],
                                ACTF.Relu, bias=lb1r_sb[:, hl:hl + 1])

                # ========== phase B: grouped vocab-sharded readout ==========
                with (
                    tc.tile_pool(name="pb", bufs=1) as pbp,
                    tc.tile_pool(name="wstream",
                                 bufs=(1 if with_vocab_bias else 2)) as wst,
                    tc.tile_pool(name="logits",
                                 bufs=(4 if with_vocab_bias else 5)) as lgp,
                    tc.tile_pool(name="sqp", bufs=2) as sqp,
                    tc.tile_pool(name="mixp", bufs=2) as mxp,
                    tc.tile_pool(name="gstat", bufs=2) as gsp,
                ):
                    if with_vocab_bias:
                        ones_sb = pbp.tile([1, 128], BF16)
                        nc.sync.dma_start(ones_sb[:], ones_d[:, :])
                        bbr_sb = pbp.tile([1, 2, VPAD], BF16)
                        nc.sync.dma_start(bbr_sb[:], bb_d[:, :, :])
                        bias_sb = pbp.tile([128, 2, VPAD], BF16)
                        for br in range(2):
                            for vc in range(NVC):
                                ps = psp.tile([128, 512], F32)
                                nc.tensor.matmul(
                                    ps[:], ones_sb[:, :],
                                    bbr_sb[:, br, vc * 512:(vc + 1) * 512],
                                    start=True, stop=True)
                                nc.scalar.activation(
                                    bias_sb[:, br, vc * 512:(vc + 1) * 512],
                                    ps[:], ACTF.Copy)

                    hsrc = [hT, h2T]
                    lg_tiles = {}
                    ar_in = []
                    ar_out = []
                    for g in range(NG):
                        ar_in.append(drp.tile([128, 24], F32, name=f"arin{g}"))
                        ar_out.append(drp.tile([128 * NCORE, 24], F32,
                                               name=f"arout{g}"))
                    rg = [list(range(NCORE))]

                    for g in range(NG):
                        for vc in range(NVC):
                            wt = wst.tile([128, 8, 2, 512], BF16, name="wt", tag="wt")
                            for kt in range(8):
                                nc.sync.dma_start(
                                    wt[:, kt, :, :],
                                    wb_d[kt * 128:(kt + 1) * 128, :,
                                         vc * 512:(vc + 1) * 512])
                            nvalid = 512 if vc < NVC - 1 else (VSH - (NVC - 1) * 512)
                            for tl in range(GSZ):
                                ti = g * GSZ + tl
                                col = ti * NVC + vc
                                if vc == 0:
                                    lg_tiles[ti] = lgp.tile(
                                        [128, 2, VPAD], BF16,
                                        name=f"lg{ti}", tag="lg")
                                lg = lg_tiles[ti]
                                for br in range(2):
                                    ps = psp.tile([128, 512], F32)
                                    for kt in range(8):
                                        nc.tensor.matmul(
                                            ps[:],
                                            hsrc[br][:, kt, ti * 128:(ti + 1) * 128],
                                            wt[:, kt, br, :],
                                            start=(kt == 0), stop=(kt == 7),
                                        )
                                    lslc = lg[:, br, vc * 512:(vc + 1) * 512]
                                    if with_vocab_bias:
                                        nc.scalar.activation(lslc, ps[:], ACTF.Copy)
                                        nc.vector.tensor_tensor(
                                            lslc, lslc,
                                            bias_sb[:, br, vc * 512:(vc + 1) * 512],
                                            ALU.add)
                                        nc.vector.tensor_reduce(
                                            ssum[br][:, col:col + 1], lslc,
                                            AXX, ALU.add)
                                    else:
                                        nc.scalar.activation(
                                            lslc, ps[:], ACTF.Copy,
                                            accum_out=ssum[br][:, col:col + 1])
                                    sq = sqp.tile([128, 512], BF16, name="sq",
                                                  tag="sq")
                                    nc.vector.tensor_tensor(sq[:], lslc, lslc,
                                                            ALU.mult)
                                    nc.vector.tensor_reduce(
                                        ssq[br][:, col:col + 1], sq[:],
                                        AXX, ALU.add)
                                    nc.vector.tensor_reduce(
                                        smax[br][:, col:col + 1],
                                        lg[:, br, vc * 512:vc * 512 + nvalid],
                                        AXX, ALU.max)

                        # ---- group stats fold + AllGather + gate + mix ----
                        st = gsp.tile([128, 24], F32, name="st", tag="st")
                        for br in range(2):
                            for tl in range(GSZ):
                                ti = g * GSZ + tl
                                sl = slice(ti * NVC, (ti + 1) * NVC)
                                nc.vector.tensor_reduce(
                                    st[:, br * 8 + tl:br * 8 + tl + 1],
                                    ssum[br][:, sl], AXX, ALU.add)
                                nc.vector.tensor_reduce(
                                    st[:, br * 8 + 4 + tl:br * 8 + 4 + tl + 1],
                                    ssq[br][:, sl], AXX, ALU.add)
                                nc.vector.tensor_reduce(
                                    st[:, 16 + br * 4 + tl:16 + br * 4 + tl + 1],
                                    smax[br][:, sl], AXX, ALU.max)
                        nc.sync.dma_start(ar_in[g][:, :], st[:])
                        nc.gpsimd.collective_compute(
                            "AllGather", ALU.bypass, replica_groups=rg,
                            ins=[ar_in[g].opt()], outs=[ar_out[g].opt()])
                        gath = gsp.tile([128, NCORE, 24], F32, name="gath",
                                        tag="gath")
                        for r in range(NCORE):
                            nc.sync.dma_start(gath[:, r, :],
                                              ar_out[g][r * 128:(r + 1) * 128, :])
                        gadd = gsp.tile([128, 16], F32, name="gadd", tag="gadd")
                        gmax = gsp.tile([128, 8], F32, name="gmax", tag="gmax")
                        nc.vector.tensor_tensor(gadd[:], gath[:, 0, 0:16],
                                                gath[:, 1, 0:16], ALU.add)
                        nc.vector.tensor_tensor(gmax[:], gath[:, 0, 16:24],
                                                gath[:, 1, 16:24], ALU.max)
                        for r in range(2, NCORE):
                            nc.vector.tensor_tensor(gadd[:], gadd[:],
                                                    gath[:, r, 0:16], ALU.add)
                            nc.vector.tensor_tensor(gmax[:], gmax[:],
                                                    gath[:, r, 16:24], ALU.max)

                        # gate features: mean, max, std per branch; [128, GSZ]
                        invV = 1.0 / float(V)
                        feats = []
                        for br in range(2):
                            mean = gsp.tile([128, GSZ], F32, name=f"mean{br}",
                                            tag=f"mean{br}")
                            nc.vector.tensor_scalar_mul(
                                mean[:], gadd[:, br * 8:br * 8 + 4], invV)
                            ms = gsp.tile([128, GSZ], F32, name=f"ms{br}",
                                          tag=f"ms{br}")
                            nc.vector.tensor_scalar_mul(
                                ms[:], gadd[:, br * 8 + 4:br * 8 + 8], invV)
                            msq = gsp.tile([128, GSZ], F32, name=f"msq{br}",
                                           tag=f"msq{br}")
                            nc.vector.tensor_tensor(msq[:], mean[:], mean[:],
                                                    ALU.mult)
                            var = gsp.tile([128, GSZ], F32, name=f"var{br}",
                                           tag=f"var{br}")
                            nc.vector.tensor_tensor(var[:], ms[:], msq[:],
                                                    ALU.subtract)
                            nc.vector.tensor_scalar_max(var[:], var[:], 0.0)
                            std = gsp.tile([128, GSZ], F32, name=f"std{br}",
                                           tag=f"std{br}")
                            nc.scalar.activation(std[:], var[:], ACTF.Sqrt)
                            feats.extend([mean[:], gmax[:, br * 4:(br + 1) * 4],
                                          std[:]])

                        acc = gsp.tile([128, GSZ], F32, name="acc", tag="acc")
                        nc.vector.tensor_scalar(acc[:], feats[0],
                                                gwb_sb[:, 0:1], None, ALU.mult)
                        for k in range(1, 6):
                            acc2 = gsp.tile([128, GSZ], F32, name=f"acc{k}",
                                            tag=f"acc{k}")
                            nc.vector.scalar_tensor_tensor(
                                acc2[:], feats[k], gwb_sb[:, k:k + 1], acc[:],
                                ALU.mult, ALU.add)
                            acc = acc2
                        nc.scalar.activation(g_sb[:, g * GSZ:(g + 1) * GSZ],
                                             acc[:], ACTF.Sigmoid,
                                             bias=gbb_sb[:, 0:1])

                        # mix + output for this group (logits still in SBUF)
                        for tl in range(GSZ):
                            ti = g * GSZ + tl
                            lg = lg_tiles.pop(ti)
                            ot = mxp.tile([128, VPAD], BF16, name="mot",
                                          tag="mot")
                            nc.vector.tensor_tensor(ot[:], lg[:, 0, :],
                                                    lg[:, 1, :], ALU.subtract)
                            nc.vector.scalar_tensor_tensor(
                                ot[:], ot[:], g_sb[:, ti:ti + 1], lg[:, 1, :],
                                ALU.mult, ALU.add)
                            nc.sync.dma_start(out_d[ti * 128:(ti + 1) * 128, :],
                                              ot[:, 0:VSH])

    nc.compile()
    return din, out_d


_CACHED = {}


def _get_program(with_vocab_bias):
    if with_vocab_bias not in _CACHED:
        nc = bacc.Bacc("TRN2", target_bir_lowering=False, debug=False,
                       num_devices=NCORE)
        build(nc, with_vocab_bias=with_vocab_bias)
        _CACHED[with_vocab_bias] = nc
    return _CACHED[with_vocab_bias]


def _prep_inputs(tokens, emb, in_proj, decays, w1, b1, w2, b2,
                 lw1, lb1, lw2, lb2, gate_w, gate_b, with_vocab_bias):
    tokens = np.asarray(tokens).astype(np.int64).reshape(-1)  # [2048]
    emb = np.asarray(emb, np.float32)
    in_proj = np.asarray(in_proj, np.float32)
    decays = np.asarray(decays, np.float32)
    w1 = np.asarray(w1, np.float32)
    b1 = np.asarray(b1, np.float32)
    lw1 = np.asarray(lw1, np.float32)
    lb1 = np.asarray(lb1, np.float32)
    w2 = np.asarray(w2, np.float32)
    b2 = np.asarray(b2, np.float32)
    lw2 = np.asarray(lw2, np.float32)
    lb2 = np.asarray(lb2, np.float32)
    gate_w = np.asarray(gate_w, np.float32).reshape(6)
    gate_b = np.asarray(gate_b, np.float32).reshape(1)

    # host-side embedding gather + transpose into the device xT layout:
    # xt[d%128, (d//128, b)] at time col 7+s  ==  emb[tokens[b*S+s], d]
    x = emb[tokens].reshape(B, S, D)                     # [2, 1024, 256]
    xt = np.zeros((128, 2, B, SP), np.float32)
    for dh in range(2):
        for b in range(B):
            xt[:, dh, b, W - 1:] = x[b, :, dh * 128:(dh + 1) * 128].T
    xt = np.ascontiguousarray(xt.reshape(128, 2 * B * SP))

    shared = {
        "xt": xt,
        "inproj": in_proj,
        "decb": np.ascontiguousarray(np.broadcast_to(decays[:, None], (M, 512))),
        "w1": w1,
        "b1r": np.ascontiguousarray(b1.reshape(8, 128).T),
        "lw1": lw1,
        "lb1r": np.ascontiguousarray(lb1.reshape(8, 128).T),
        "gwb": np.ascontiguousarray(np.broadcast_to(gate_w[None, :], (128, 6))),
        "gbb": np.full((128, 1), gate_b[0], np.float32),
    }
    if with_vocab_bias:
        shared["ones"] = np.ones((1, 128), ml_dtypes.bfloat16)

    in_maps = []
    for c in range(NCORE):
        sl = slice(c * VSH, (c + 1) * VSH)
        wb = np.zeros((HL, 2, VPAD), ml_dtypes.bfloat16)
        wb[:, 0, :VSH] = w2[:, sl].astype(ml_dtypes.bfloat16)
        wb[:, 1, :VSH] = lw2[:, sl].astype(ml_dtypes.bfloat16)
        m = dict(shared)
        m["wb"] = wb
        if with_vocab_bias:
            bb = np.zeros((1, 2, VPAD), ml_dtypes.bfloat16)
            bb[0, 0, :VSH] = b2[sl].astype(ml_dtypes.bfloat16)
            bb[0, 1, :VSH] = lb2[sl].astype(ml_dtypes.bfloat16)
            m["bb"] = bb
        in_maps.append(m)
    return in_maps


def kernel(**inputs):
    global LAST_RESULT
    with_vocab_bias = bool(np.any(np.asarray(inputs["b2"]))
                           or np.any(np.asarray(inputs["lb2"])))
    nc = _get_program(with_vocab_bias)
    in_maps = _prep_inputs(**inputs, with_vocab_bias=with_vocab_bias)
    res = run_bass_kernel_spmd(nc, in_maps, list(range(NCORE)))
    LAST_RESULT = res
    full = np.empty((B, S, V), np.float32)
    for c in range(NCORE):
        full[:, :, c * VSH:(c + 1) * VSH] = (
            res.results[c]["out"].astype(np.float32).reshape(B, S, VSH))
    return full


# revision 14
# speedup vs baseline: 1.1380x; 1.1380x over previous
"""Trainium2 Bass kernel for nn_CausalBankModel (decay-bank LM head).

Strategy (8 NeuronCores, vocab-tensor-parallel):
  - Every core computes the shared trunk (embedding gather on host, mode
    projection, decay-bank scan, both hidden layers) redundantly (~25% of
    the FLOPs); the two big [2048,1024]@[1024,4096] readout matmuls are
    sharded over vocab: core c owns columns [c*4000, (c+1)*4000).
  - Readout is processed in 4 groups of 4 token-tiles. Logit tiles stay
    RESIDENT IN SBUF (no DRAM round-trip). Per-group partial gate stats
    (sum/sumsq/max over the local vocab slice) are exchanged with one
    small AllGather per group; the gate + mixture for that group then
    runs from SBUF, overlapped with the next group's matmuls.
  - Stats work is split between ScalarE (PSUM->SBUF copy + sum accum) and
    VectorE (square+sumsq, max) to keep both far below TensorE's span.

Layouts on device (partition dim first):
  - xT    : [128(d), dh, b, 7+S] transposed embeddings, 7 zero cols pad
  - statesT: [128(m), mt, b, S]  decay-bank states (tensor_tensor_scan)
  - hT/h2T: [128(hidden), kt, B*S] bf16, lhsT of the readout matmuls
  - logits: [128(token), 2(br), 4096(vocab)] bf16, SBUF-resident per tile
"""

import os
import sys

import numpy as np

for _p in ("/opt/trn_rl_repo", "/opt/pypackages"):
    if _p not in sys.path and os.path.isdir(_p):
        sys.path.append(_p)

import ml_dtypes  # noqa: E402

from concourse import bacc, bass, tile  # noqa: E402
from concourse import mybir  # noqa: E402
from concourse.bass_utils import run_bass_kernel_spmd  # noqa: E402

F32 = mybir.dt.float32
F32R = mybir.dt.float32r
BF16 = mybir.dt.bfloat16
ALU = mybir.AluOpType
ACTF = mybir.ActivationFunctionType
AXX = mybir.AxisListType.X

V = 32000
D = 256
M = 256
W = 8
HL = 1024  # hidden width (both branches)
B = 2
S = 1024
BS = B * S            # 2048 tokens
NCORE = 8
VSH = V // NCORE      # 4000 true vocab cols per core
VPAD = 4096           # padded slice width
NVC = VPAD // 512     # 8 vocab chunks of 512
NT = BS // 128        # 16 token tiles
SP = S + W - 1        # 1031, padded time length
GSZ = 4               # token tiles per stats group
NG = NT // GSZ        # 4 groups

LAST_RESULT = None


def build(nc, with_vocab_bias=True):
    din = {}

    def inp(name, shape, dt):
        din[name] = nc.dram_tensor(name, list(shape), dt, kind="ExternalInput")
        return din[name]

    xt_d = inp("xt", [128, 2 * B * SP], F32R)
    inproj_d = inp("inproj", [D, M], F32R)
    decb_d = inp("decb", [M, 512], F32)
    w1_d = inp("w1", [M + D, HL], F32R)
    b1r_d = inp("b1r", [128, HL // 128], F32)
    lw1_d = inp("lw1", [W * D, HL], F32R)
    lb1r_d = inp("lb1r", [128, HL // 128], F32)
    wb_d = inp("wb", [HL, 2, VPAD], BF16)     # stacked [w2 | lw2]
    gwb_d = inp("gwb", [128, 6], F32)
    gbb_d = inp("gbb", [128, 1], F32)
    if with_vocab_bias:
        bb_d = inp("bb", [1, 2, VPAD], BF16)  # stacked [b2 | lb2]
        ones_d = inp("ones", [1, 128], BF16)

    out_d = nc.dram_tensor("out", [BS, VSH], BF16, kind="ExternalOutput")

    with tile.TileContext(nc) as tc:
        with (
            tc.tile_pool(name="cst", bufs=1) as cst,
            tc.tile_pool(name="ps", bufs=8, space=bass.MemorySpace.PSUM) as psp,
            tc.tile_pool(name="dram", bufs=1, space="DRAM") as drp,
        ):
            # per-(ti,vc) raw stats + gate consts (tiny, long-lived)
            # warmup collective: absorbs the ~200us one-time comm init (and
            # inter-core start skew) during the trunk, so the first real
            # stats AllGather runs at its ~6us floor.
            wu_sb = cst.tile([128, 1], F32)
            nc.vector.memset(wu_sb[:], 0.0)
            wu_in = drp.tile([128, 1], F32, name="wuin")
            wu_out = drp.tile([128 * NCORE, 1], F32, name="wuout")
            nc.sync.dma_start(wu_in[:, :], wu_sb[:])
            nc.gpsimd.collective_compute(
                "AllGather", ALU.bypass, replica_groups=[list(range(NCORE))],
                ins=[wu_in.opt()], outs=[wu_out.opt()])

            ssum = [cst.tile([128, NT * NVC], F32, name=f"ssum{i}") for i in range(2)]
            ssq = [cst.tile([128, NT * NVC], F32, name=f"ssq{i}") for i in range(2)]
            smax = [cst.tile([128, NT * NVC], BF16, name=f"smax{i}") for i in range(2)]
            gwb_sb = cst.tile([128, 6], F32)
            nc.sync.dma_start(gwb_sb[:], gwb_d[:, :])
            gbb_sb = cst.tile([128, 1], F32)
            nc.sync.dma_start(gbb_sb[:], gbb_d[:, :])
            g_sb = cst.tile([128, NT], F32)
            gm1_sb = cst.tile([128, NT], F32)

            with tc.tile_pool(name="ph", bufs=1) as php:  # spans trunk + readout
                hT = php.tile([128, 8, BS], BF16)
                h2T = php.tile([128, 8, BS], BF16)

                # ================= phase A: shared trunk =================
                with tc.tile_pool(name="pa", bufs=1) as pap:
                    xT = pap.tile([128, 2, B, SP], F32R)
                    for dh in range(2):
                        for b in range(B):
                            nc.sync.dma_start(
                                xT[:, dh, b, :],
                                xt_d[:, (dh * B + b) * SP:(dh * B + b + 1) * SP])

                    inproj_sb = pap.tile([128, 2, M], F32R)
                    for kt in range(2):
                        nc.sync.dma_start(inproj_sb[:, kt, :],
                                          inproj_d[kt * 128:(kt + 1) * 128, :])
                    decb_sb = pap.tile([128, 2, 512], F32)
                    for mt in range(2):
                        nc.sync.dma_start(decb_sb[:, mt, :],
                                          decb_d[mt * 128:(mt + 1) * 128, :])
                    b1r_sb = pap.tile([128, 8], F32)
                    nc.sync.dma_start(b1r_sb[:], b1r_d[:, :])
                    lb1r_sb = pap.tile([128, 8], F32)
                    nc.sync.dma_start(lb1r_sb[:], lb1r_d[:, :])
                    w1_sb = pap.tile([128, 4, HL], F32R)
                    for kt in range(4):
                        nc.sync.dma_start(w1_sb[:, kt, :],
                                          w1_d[kt * 128:(kt + 1) * 128, :])
                    # hoisted so the 8MB load overlaps the scan + w1 matmuls
                    lw1_sb = pap.tile([128, 16, HL], F32R)
                    for kt in range(16):
                        nc.sync.dma_start(lw1_sb[:, kt, :],
                                          lw1_d[kt * 128:(kt + 1) * 128, :])

                    statesT = pap.tile([128, 2, B, S], F32R)
                    for mt in range(2):
                        for b in range(B):
                            for hf in range(2):
                                ps = psp.tile([128, 512], F32)
                                for kt in range(2):
                                    nc.tensor.matmul(
                                        ps[:],
                                        inproj_sb[:, kt, mt * 128:(mt + 1) * 128],
                                        xT[:, kt, b,
                                           W - 1 + hf * 512:W - 1 + hf * 512 + 512],
                                        start=(kt == 0), stop=(kt == 1),
                                    )
                                init = (0.0 if hf == 0 else
                                        statesT[:, mt, b, hf * 512 - 1:hf * 512])
                                nc.vector.tensor_tensor_scan(
                                    statesT[:, mt, b, hf * 512:hf * 512 + 512],
                                    decb_sb[:, mt, :], ps[:], init,
                                    ALU.mult, ALU.add,
                                )

                    for hl in range(8):
                        for ch in range(4):
                            b, hf = ch // 2, ch % 2
                            ps = psp.tile([128, 512], F32)
                            for kt in range(4):
                                if kt < 2:
                                    rhs = statesT[:, kt, b, hf * 512:hf * 512 + 512]
                                else:
                                    rhs = xT[:, kt - 2, b,
                                             W - 1 + hf * 512:W - 1 + hf * 512 + 512]
                                nc.tensor.matmul(
                                    ps[:], w1_sb[:, kt, hl * 128:(hl + 1) * 128],
                                    rhs, start=(kt == 0), stop=(kt == 3),
                                )
                            nc.scalar.activation(
                                hT[:, hl, ch * 512:(ch + 1) * 512], ps[:],
                                ACTF.Relu, bias=b1r_sb[:, hl:hl + 1])

                    for hl in range(8):
                        for ch in range(4):
                            b, hf = ch // 2, ch % 2
                            ps = psp.tile([128, 512], F32)
                            for ki in range(16):
                                w, dh = ki // 2, ki % 2
                                rhs = xT[:, dh, b, hf * 512 + w:hf * 512 + w + 512]
                                nc.tensor.matmul(
                                    ps[:], lw1_sb[:, ki, hl * 128:(hl + 1) * 128],
                                    rhs, start=(ki == 0), stop=(ki == 15),
                                )
                            nc.scalar.activation(
                                h2T[:, hl, ch * 512:(ch + 1) * 512], ps[:],
                                ACTF.Relu, bias=lb1r_sb[:, hl:hl + 1])

                # ========== phase B: grouped vocab-sharded readout ==========
                with (
                    tc.tile_pool(name="pb", bufs=1) as pbp,
                    tc.tile_pool(name="wstream",
                                 bufs=(1 if with_vocab_bias else 2)) as wst,
                    tc.tile_pool(name="logits",
                                 bufs=(4 if with_vocab_bias else 5)) as lgp,
                    tc.tile_pool(name="sqp", bufs=2) as sqp,
                    tc.tile_pool(name="mixp", bufs=2) as mxp,
                    tc.tile_pool(name="mix2", bufs=1) as mx2,
                    tc.tile_pool(name="gstat", bufs=2) as gsp,
                ):
                    if with_vocab_bias:
                        ones_sb = pbp.tile([1, 128], BF16)
                        nc.sync.dma_start(ones_sb[:], ones_d[:, :])
                        bbr_sb = pbp.tile([1, 2, VPAD], BF16)
                        nc.sync.dma_start(bbr_sb[:], bb_d[:, :, :])
                        bias_sb = pbp.tile([128, 2, VPAD], BF16)
                        for br in range(2):
                            for vc in range(NVC):
                                ps = psp.tile([128, 512], F32)
                                nc.tensor.matmul(
                                    ps[:], ones_sb[:, :],
                                    bbr_sb[:, br, vc * 512:(vc + 1) * 512],
                                    start=True, stop=True)
                                nc.scalar.activation(
                                    bias_sb[:, br, vc * 512:(vc + 1) * 512],
                                    ps[:], ACTF.Copy)

                    hsrc = [hT, h2T]
                    lg_tiles = {}
                    ar_in = []
                    ar_out = []
                    for g in range(NG):
                        ar_in.append(drp.tile([128, 24], F32, name=f"arin{g}"))
                        ar_out.append(drp.tile([128 * NCORE, 24], F32,
                                               name=f"arout{g}"))
                    rg = [list(range(NCORE))]

                    for g in range(NG):
                        for vc in range(NVC):
                            wt = wst.tile([128, 8, 2, 512], BF16, name="wt", tag="wt")
                            for kt in range(8):
                                nc.sync.dma_start(
                                    wt[:, kt, :, :],
                                    wb_d[kt * 128:(kt + 1) * 128, :,
                                         vc * 512:(vc + 1) * 512])
                            nvalid = 512 if vc < NVC - 1 else (VSH - (NVC - 1) * 512)
                            for tl in range(GSZ):
                                ti = g * GSZ + tl
                                col = ti * NVC + vc
                                if vc == 0:
                                    lg_tiles[ti] = lgp.tile(
                                        [128, 2, VPAD], BF16,
                                        name=f"lg{ti}", tag="lg")
                                lg = lg_tiles[ti]
                                for br in range(2):
                                    ps = psp.tile([128, 512], F32)
                                    for kt in range(8):
                                        nc.tensor.matmul(
                                            ps[:],
                                            hsrc[br][:, kt, ti * 128:(ti + 1) * 128],
                                            wt[:, kt, br, :],
                                            start=(kt == 0), stop=(kt == 7),
                                        )
                                    lslc = lg[:, br, vc * 512:(vc + 1) * 512]
                                    if with_vocab_bias:
                                        nc.scalar.activation(lslc, ps[:], ACTF.Copy)
                                        nc.vector.tensor_tensor(
                                            lslc, lslc,
                                            bias_sb[:, br, vc * 512:(vc + 1) * 512],
                                            ALU.add)
                                        nc.vector.tensor_reduce(
                                            ssum[br][:, col:col + 1], lslc,
                                            AXX, ALU.add)
                                    else:
                                        nc.scalar.activation(
                                            lslc, ps[:], ACTF.Copy,
                                            accum_out=ssum[br][:, col:col + 1])
                                    sq = sqp.tile([128, 512], BF16, name="sq",
                                                  tag="sq")
                                    nc.vector.tensor_tensor(sq[:], lslc, lslc,
                                                            ALU.mult)
                                    nc.vector.tensor_reduce(
                                        ssq[br][:, col:col + 1], sq[:],
                                        AXX, ALU.add)
                                    nc.vector.tensor_reduce(
                                        smax[br][:, col:col + 1],
                                        lg[:, br, vc * 512:vc * 512 + nvalid],
                                        AXX, ALU.max)

                        # ---- group stats fold + AllGather + gate + mix ----
                        st = gsp.tile([128, 24], F32, name="st", tag="st")
                        for br in range(2):
                            for tl in range(GSZ):
                                ti = g * GSZ + tl
                                sl = slice(ti * NVC, (ti + 1) * NVC)
                                nc.vector.tensor_reduce(
                                    st[:, br * 8 + tl:br * 8 + tl + 1],
                                    ssum[br][:, sl], AXX, ALU.add)
                                nc.vector.tensor_reduce(
                                    st[:, br * 8 + 4 + tl:br * 8 + 4 + tl + 1],
                                    ssq[br][:, sl], AXX, ALU.add)
                                nc.vector.tensor_reduce(
                                    st[:, 16 + br * 4 + tl:16 + br * 4 + tl + 1],
                                    smax[br][:, sl], AXX, ALU.max)
                        nc.sync.dma_start(ar_in[g][:, :], st[:])
                        nc.gpsimd.collective_compute(
                            "AllGather", ALU.bypass, replica_groups=rg,
                            ins=[ar_in[g].opt()], outs=[ar_out[g].opt()])
                        gath = gsp.tile([128, NCORE, 24], F32, name="gath",
                                        tag="gath")
                        for r in range(NCORE):
                            nc.sync.dma_start(gath[:, r, :],
                                              ar_out[g][r * 128:(r + 1) * 128, :])
                        gadd = gsp.tile([128, 16], F32, name="gadd", tag="gadd")
                        gmax = gsp.tile([128, 8], F32, name="gmax", tag="gmax")
                        nc.vector.tensor_tensor(gadd[:], gath[:, 0, 0:16],
                                                gath[:, 1, 0:16], ALU.add)
                        nc.vector.tensor_tensor(gmax[:], gath[:, 0, 16:24],
                                                gath[:, 1, 16:24], ALU.max)
                        for r in range(2, NCORE):
                            nc.vector.tensor_tensor(gadd[:], gadd[:],
                                                    gath[:, r, 0:16], ALU.add)
                            nc.vector.tensor_tensor(gmax[:], gmax[:],
                                                    gath[:, r, 16:24], ALU.max)

                        # gate features: mean, max, std per branch; [128, GSZ]
                        invV = 1.0 / float(V)
                        feats = []
                        for br in range(2):
                            mean = gsp.tile([128, GSZ], F32, name=f"mean{br}",
                                            tag=f"mean{br}")
                            nc.vector.tensor_scalar_mul(
                                mean[:], gadd[:, br * 8:br * 8 + 4], invV)
                            ms = gsp.tile([128, GSZ], F32, name=f"ms{br}",
                                          tag=f"ms{br}")
                            nc.vector.tensor_scalar_mul(
                                ms[:], gadd[:, br * 8 + 4:br * 8 + 8], invV)
                            msq = gsp.tile([128, GSZ], F32, name=f"msq{br}",
                                           tag=f"msq{br}")
                            nc.vector.tensor_tensor(msq[:], mean[:], mean[:],
                                                    ALU.mult)
                            var = gsp.tile([128, GSZ], F32, name=f"var{br}",
                                           tag=f"var{br}")
                            nc.vector.tensor_tensor(var[:], ms[:], msq[:],
                                                    ALU.subtract)
                            nc.vector.tensor_scalar_max(var[:], var[:], 0.0)
                            std = gsp.tile([128, GSZ], F32, name=f"std{br}",
                                           tag=f"std{br}")
                            nc.scalar.activation(std[:], var[:], ACTF.Sqrt)
                            feats.extend([mean[:], gmax[:, br * 4:(br + 1) * 4],
                                          std[:]])

                        acc = gsp.tile([128, GSZ], F32, name="acc", tag="acc")
                        nc.vector.tensor_scalar(acc[:], feats[0],
                                                gwb_sb[:, 0:1], None, ALU.mult)
                        for k in range(1, 6):
                            acc2 = gsp.tile([128, GSZ], F32, name=f"acc{k}",
                                            tag=f"acc{k}")
                            nc.vector.scalar_tensor_tensor(
                                acc2[:], feats[k], gwb_sb[:, k:k + 1], acc[:],
                                ALU.mult, ALU.add)
                            acc = acc2
                        nc.scalar.activation(g_sb[:, g * GSZ:(g + 1) * GSZ],
                                             acc[:], ACTF.Sigmoid,
                                             bias=gbb_sb[:, 0:1])
                        nc.vector.tensor_scalar(gm1_sb[:, g * GSZ:(g + 1) * GSZ],
                                                g_sb[:, g * GSZ:(g + 1) * GSZ],
                                                -1.0, 1.0, ALU.mult, ALU.add)

                        # mix + output for this group (logits still in SBUF);
                        # lg slot frees as soon as the two tensor_scalar
                        # reads complete, so the next group's matmuls don't
                        # stall on the gate/output chain.
                        for tl in range(GSZ):
                            ti = g * GSZ + tl
                            lg = lg_tiles.pop(ti)
                            t1 = mxp.tile([128, VPAD], BF16, name="mt1",
                                          tag="mt1")
                            nc.vector.tensor_scalar(t1[:], lg[:, 0, :],
                                                    g_sb[:, ti:ti + 1], None,
                                                    ALU.mult)
                            t2 = mx2.tile([128, VPAD], BF16, name="mt2",
                                          tag="mt2")
                            nc.vector.tensor_scalar(t2[:], lg[:, 1, :],
                                                    gm1_sb[:, ti:ti + 1], None,
                                                    ALU.mult)
                            nc.vector.tensor_tensor(t1[:], t1[:], t2[:],
                                                    ALU.add)
                            nc.sync.dma_start(out_d[ti * 128:(ti + 1) * 128, :],
                                              t1[:, 0:VSH])

    nc.compile()
    return din, out_d


_CACHED = {}


def _get_program(with_vocab_bias):
    if with_vocab_bias not in _CACHED:
        nc = bacc.Bacc("TRN2", target_bir_lowering=False, debug=False,
                       num_devices=NCORE)
        build(nc, with_vocab_bias=with_vocab_bias)
        _CACHED[with_vocab_bias] = nc
    return _CACHED[with_vocab_bias]


def _prep_inputs(tokens, emb, in_proj, decays, w1, b1, w2, b2,
                 lw1, lb1, lw2, lb2, gate_w, gate_b, with_vocab_bias):
    tokens = np.asarray(tokens).astype(np.int64).reshape(-1)  # [2048]
    emb = np.asarray(emb, np.float32)
    in_proj = np.asarray(in_proj, np.float32)
    decays = np.asarray(decays, np.float32)
    w1 = np.asarray(w1, np.float32)
    b1 = np.asarray(b1, np.float32)
    lw1 = np.asarray(lw1, np.float32)
    lb1 = np.asarray(lb1, np.float32)
    w2 = np.asarray(w2, np.float32)
    b2 = np.asarray(b2, np.float32)
    lw2 = np.asarray(lw2, np.float32)
    lb2 = np.asarray(lb2, np.float32)
    gate_w = np.asarray(gate_w, np.float32).reshape(6)
    gate_b = np.asarray(gate_b, np.float32).reshape(1)

    # host-side embedding gather + transpose into the device xT layout:
    # xt[d%128, (d//128, b)] at time col 7+s  ==  emb[tokens[b*S+s], d]
    x = emb[tokens].reshape(B, S, D)                     # [2, 1024, 256]
    xt = np.zeros((128, 2, B, SP), np.float32)
    for dh in range(2):
        for b in range(B):
            xt[:, dh, b, W - 1:] = x[b, :, dh * 128:(dh + 1) * 128].T
    xt = np.ascontiguousarray(xt.reshape(128, 2 * B * SP))

    shared = {
        "xt": xt,
        "inproj": in_proj,
        "decb": np.ascontiguousarray(np.broadcast_to(decays[:, None], (M, 512))),
        "w1": w1,
        "b1r": np.ascontiguousarray(b1.reshape(8, 128).T),
        "lw1": lw1,
        "lb1r": np.ascontiguousarray(lb1.reshape(8, 128).T),
        "gwb": np.ascontiguousarray(np.broadcast_to(gate_w[None, :], (128, 6))),
        "gbb": np.full((128, 1), gate_b[0], np.float32),
    }
    if with_vocab_bias:
        shared["ones"] = np.ones((1, 128), ml_dtypes.bfloat16)

    in_maps = []
    for c in range(NCORE):
        sl = slice(c * VSH, (c + 1) * VSH)
        wb = np.zeros((HL, 2, VPAD), ml_dtypes.bfloat16)
        wb[:, 0, :VSH] = w2[:, sl].astype(ml_dtypes.bfloat16)
        wb[:, 1, :VSH] = lw2[:, sl].astype(ml_dtypes.bfloat16)
        m = dict(shared)
        m["wb"] = wb
        if with_vocab_bias:
            bb = np.zeros((1, 2, VPAD), ml_dtypes.bfloat16)
            bb[0, 0, :VSH] = b2[sl].astype(ml_dtypes.bfloat16)
            bb[0, 1, :VSH] = lb2[sl].astype(ml_dtypes.bfloat16)
            m["bb"] = bb
        in_maps.append(m)
    return in_maps


def kernel(**inputs):
    global LAST_RESULT
    with_vocab_bias = bool(np.any(np.asarray(inputs["b2"]))
                           or np.any(np.asarray(inputs["lb2"])))
    nc = _get_program(with_vocab_bias)
    in_maps = _prep_inputs(**inputs, with_vocab_bias=with_vocab_bias)
    res = run_bass_kernel_spmd(nc, in_maps, list(range(NCORE)))
    LAST_RESULT = res
    full = np.empty((B, S, V), np.float32)
    for c in range(NCORE):
        full[:, :, c * VSH:(c + 1) * VSH] = (
            res.results[c]["out"].astype(np.float32).reshape(B, S, VSH))
    return full
